# revision 1
# baseline (speedup 1.0000x reference)
"""Self-contained Trainium2 Bass kernel for the CRF forward-algorithm problem.

Model (see problem statement):
    A = exp(WA with col BOS=-inf)                       [64, 64]
    emit(word) = exp(ThetaB @ E[word])                  [64] per word, rows 62,63 -> ~0
    per sentence: forward recursion over 126 steps, twice (tagged one-hot
    mask / unsupervised), output = logZ_tagged - logZ_unsup      [512] f32

Strategy (8 cores, data-parallel over sentences, 64 sentences/core):
  - Host: cast E to bf16, pre-transpose ThetaB, precompute A; flatten the
    per-core word/tag indices in "instance" order c = (t-1)*64 + s.
  - Device phase 1 (per 128-instance chunk = 2 time steps):
      indirect-DMA gather of E rows (bf16, 1KB each) -> Eg [128, 512]
      transpose to EgT [128d, 128inst] x4 (DMA-transpose bf16, or PE transpose)
      wb = ThetaB @ Eg.T via 4 accumulated matmuls -> PSUM [64, 128]
      ACT exp (+row bias -1000 on rows 62,63) -> ES unsup cols;
      one-hot mask (outer-product + is_equal) -> tagged cols
  - Device phase 2: batched recursion alpha[64 tags, 128 seq-cols]
      (64 tagged | 64 unsup), per step: PSUM = A.T @ alpha (PE), then
      alpha = PSUM * ES_t (DVE). No per-step normalization; every 10 steps
      rescale columns by 1/colsum (measured at t=2g, folded into the ES slice
      of step 2g+2, off the critical path) and accumulate log(colsum).
  - Final: dot with A[:, EOS], log, add logacc, subtract tagged-unsup.
"""

import numpy as np

K = 64
V = 100000
D = 512
B = 512
T = 128
BOS_T = K - 1  # 63
EOS_T = K - 2  # 62
NCORES = 8
S = B // NCORES  # 64 sentences per core
W2 = 2 * S  # 128 virtual sequences (tagged | unsup)

# "dma" = bf16 DMA-transpose path, "pe" = PE transpose + PSUM->SBUF copy path
TRANSPOSE_MODE = "dma"
RESCALE_G = 5  # rescale every RESCALE_G chunks (= 2*RESCALE_G steps)


def build_bass(T_=T, V_=V, transpose_mode=TRANSPOSE_MODE, reps=1):
    import concourse.bass as bass
    import concourse.bacc as bacc
    import concourse.mybir as mybir
    import concourse.tile as tile
    from concourse.masks import make_identity

    steps = T_ - 2
    assert steps % 2 == 0
    nchunks = steps // 2
    ni = steps * S  # instances per core

    f32 = mybir.dt.float32
    bf16 = mybir.dt.bfloat16
    i32 = mybir.dt.int32

    nc = bacc.Bacc(None)

    # ---- I/O ----
    E_d = nc.dram_tensor("E", [V_, D], bf16, kind="ExternalInput")
    TBT_d = nc.dram_tensor("TBT", [D, K], bf16, kind="ExternalInput")  # ThetaB.T
    A_d = nc.dram_tensor("Amat", [K, K], f32, kind="ExternalInput")
    Aeos_d = nc.dram_tensor("Aeos", [K, 1], f32, kind="ExternalInput")
    widx_d = nc.dram_tensor("widx", [128, nchunks], i32, kind="ExternalInput")
    tagsf_d = nc.dram_tensor("tagsf", [1, ni], f32, kind="ExternalInput")
    iota_d = nc.dram_tensor("iota64", [K, 1], f32, kind="ExternalInput")
    # per-partition exp bias: 0 for tag rows, -1000 for rows 62,63 (exp -> 0)
    ebias_d = nc.dram_tensor("ebias", [K, 1], f32, kind="ExternalInput")
    alpha0_d = nc.dram_tensor("alpha0", [K, 1], f32, kind="ExternalInput")
    out_d = nc.dram_tensor("out", [1, S], f32, kind="ExternalOutput")

    with tile.TileContext(nc) as tc:
        with (
            tc.tile_pool(name="const", bufs=1) as cpool,
            tc.tile_pool(name="eg", bufs=3) as egpool,
            tc.tile_pool(name="egt", bufs=3) as egtpool,
            tc.tile_pool(name="es", bufs=nchunks) as espool,
            tc.tile_pool(name="mask", bufs=2) as maskpool,
            tc.tile_pool(name="small", bufs=4) as smallpool,
            tc.tile_pool(name="ps_tp", bufs=2, space="PSUM") as ps_tp,
            tc.tile_pool(name="ps_wb", bufs=2, space="PSUM") as ps_wb,
            tc.tile_pool(name="ps_rec", bufs=2, space="PSUM") as ps_rec,
            tc.tile_pool(name="ps_misc", bufs=2, space="PSUM") as ps_misc,
        ):
            # ---- constants / persistent state ----
            tbt_sb = cpool.tile([128, 4, K], bf16)  # TBT d-chunk j at [:, j, :]
            nc.sync.dma_start(tbt_sb[:], TBT_d[:].rearrange("(c p) k -> p c k", p=128))
            amat_sb = cpool.tile([K, K], f32)
            nc.sync.dma_start(amat_sb[:], A_d[:])
            aeos_sb = cpool.tile([K, 1], f32)
            nc.sync.dma_start(aeos_sb[:], Aeos_d[:])
            iota_sb = cpool.tile([K, 1], f32)
            nc.sync.dma_start(iota_sb[:], iota_d[:])
            ebias_sb = cpool.tile([K, 1], f32)
            nc.sync.dma_start(ebias_sb[:], ebias_d[:])
            alpha0_sb = cpool.tile([K, 1], f32)
            nc.sync.dma_start(alpha0_sb[:], alpha0_d[:])
            widx_sb = cpool.tile([128, nchunks], i32)
            nc.sync.dma_start(widx_sb[:], widx_d[:])
            tagsf_sb = cpool.tile([1, ni], f32)
            nc.sync.dma_start(tagsf_sb[:], tagsf_d[:])

            ones_k1 = cpool.tile([K, 1], f32)  # column of ones (colsum lhsT)
            nc.vector.memset(ones_k1[:], 1.0)
            ones_1k = cpool.tile([1, K], f32)  # row of ones (outer-product lhsT)
            nc.vector.memset(ones_1k[:], 1.0)

            if transpose_mode == "pe":
                ident_sb = cpool.tile([128, 128], bf16)
                make_identity(nc, ident_sb[:])

            alpha = cpool.tile([K, W2], f32)
            logacc = cpool.tile([1, W2], f32)

            for _rep in range(reps):
                nc.vector.tensor_copy(alpha[:], alpha0_sb[:].to_broadcast([K, W2]))
                nc.vector.memset(logacc[:], 0.0)

                for g in range(nchunks):
                    # ------- phase 1: emissions for steps 2g+1, 2g+2 -------
                    eg = egpool.tile([128, D], bf16, tag="eg")
                    nc.gpsimd.indirect_dma_start(
                        out=eg[:],
                        out_offset=None,
                        in_=E_d[:],
                        in_offset=bass.IndirectOffsetOnAxis(
                            ap=widx_sb[:, g : g + 1], axis=0
                        ),
                    )

                    egts = [
                        egtpool.tile(
                            [128, 128], bf16, tag=f"egt{j}", name=f"egt{j}_{g}"
                        )
                        for j in range(4)
                    ]
                    if transpose_mode == "dma":
                        for j in range(4):
                            nc.sync.dma_start(
                                egts[j][:],
                                eg[:, j * 128 : (j + 1) * 128],
                                transpose=True,
                            )
                    else:
                        for j in range(4):
                            tp = ps_tp.tile([128, 128], bf16, tag="tp")
                            nc.tensor.transpose(
                                out=tp[:],
                                in_=eg[:, j * 128 : (j + 1) * 128],
                                identity=ident_sb[:],
                            )
                            nc.vector.tensor_copy(egts[j][:], tp[:])

                    wb = ps_wb.tile([K, W2], f32, tag="wb")
                    for j in range(4):
                        nc.tensor.matmul(
                            wb[:],
                            lhsT=tbt_sb[:, j, :],
                            rhs=egts[j][:],
                            start=(j == 0),
                            stop=(j == 3),
                        )

                    # ES layout [64, 4*S]: [tagA | unA | tagB | unB]
                    es = espool.tile([K, 4 * S], f32, tag="es")
                    # unsup cols: exp(wb + ebias); rows 62,63 get bias -> 0
                    nc.scalar.activation(
                        es[:, 1 * S : 2 * S],
                        wb[:, 0:S],
                        mybir.ActivationFunctionType.Exp,
                        bias=ebias_sb[:],
                    )
                    nc.scalar.activation(
                        es[:, 3 * S : 4 * S],
                        wb[:, S : 2 * S],
                        mybir.ActivationFunctionType.Exp,
                        bias=ebias_sb[:],
                    )

                    # one-hot mask for the tagged halves
                    tago = ps_misc.tile([K, W2], f32, tag="misc")
                    nc.tensor.matmul(
                        tago[:],
                        lhsT=ones_1k[:],
                        rhs=tagsf_sb[:, g * W2 : (g + 1) * W2],
                        start=True,
                        stop=True,
                    )
                    msk = maskpool.tile([K, W2], f32, tag="msk")
                    nc.vector.tensor_scalar(
                        msk[:], tago[:], iota_sb[:], None, mybir.AluOpType.is_equal
                    )
                    # tagged cols = unsup cols * mask (rows 62,63 -> 0)
                    nc.vector.tensor_tensor(
                        es[:, 0 * S : 1 * S],
                        es[:, 1 * S : 2 * S],
                        msk[:, 0:S],
                        mybir.AluOpType.mult,
                    )
                    nc.vector.tensor_tensor(
                        es[:, 2 * S : 3 * S],
                        es[:, 3 * S : 4 * S],
                        msk[:, S : 2 * S],
                        mybir.AluOpType.mult,
                    )

                    # ------- rescale (every RESCALE_G chunks) -------
                    if g > 0 and g % RESCALE_G == 0:
                        colsum = ps_misc.tile([1, W2], f32, tag="misc")
                        nc.tensor.matmul(
                            colsum[:],
                            lhsT=ones_k1[:],
                            rhs=alpha[:],
                            start=True,
                            stop=True,
                        )
                        recip = smallpool.tile([1, W2], f32, tag="recip")
                        nc.vector.reciprocal(recip[:], colsum[:])
                        ltmp = smallpool.tile([1, W2], f32, tag="ltmp")
                        nc.scalar.activation(
                            ltmp[:], colsum[:], mybir.ActivationFunctionType.Ln
                        )
                        nc.vector.tensor_tensor(
                            logacc[:], logacc[:], ltmp[:], mybir.AluOpType.add
                        )
                        rbc = ps_misc.tile([K, W2], f32, tag="misc")
                        nc.tensor.matmul(
                            rbc[:], lhsT=ones_1k[:], rhs=recip[:], start=True, stop=True
                        )
                        # fold scale into the ES slice of step 2g+2
                        nc.vector.tensor_tensor(
                            es[:, 2 * S : 4 * S],
                            es[:, 2 * S : 4 * S],
                            rbc[:],
                            mybir.AluOpType.mult,
                        )

                    # ------- phase 2: recursion steps 2g+1, 2g+2 -------
                    for half in range(2):
                        rec = ps_rec.tile([K, W2], f32, tag="rec")
                        nc.tensor.matmul(
                            rec[:],
                            lhsT=amat_sb[:],
                            rhs=alpha[:],
                            start=True,
                            stop=True,
                        )
                        nc.vector.tensor_tensor(
                            alpha[:],
                            rec[:],
                            es[:, half * W2 : (half + 1) * W2],
                            mybir.AluOpType.mult,
                        )

                # ------- final -------
                eos = ps_misc.tile([1, W2], f32, tag="misc")
                nc.tensor.matmul(
                    eos[:], lhsT=aeos_sb[:], rhs=alpha[:], start=True, stop=True
                )
                ltot = smallpool.tile([1, W2], f32, tag="ltot")
                nc.scalar.activation(ltot[:], eos[:], mybir.ActivationFunctionType.Ln)
                nc.vector.tensor_tensor(
                    ltot[:], ltot[:], logacc[:], mybir.AluOpType.add
                )
                res = smallpool.tile([1, S], f32, tag="res")
                nc.vector.tensor_tensor(
                    res[:], ltot[:, 0:S], ltot[:, S:W2], mybir.AluOpType.subtract
                )
                nc.sync.dma_start(out_d[:], res[:])

    nc.compile()
    return nc


def make_in_maps(WA, ThetaB, E, words, tags, T_=T, V_=V):
    import ml_dtypes

    steps = T_ - 2
    nchunks = steps // 2
    ni = steps * S

    WAm = np.array(WA, np.float32).copy()
    WAm[:, BOS_T] = -np.inf
    A = np.exp(WAm).astype(np.float32)
    Aeos = np.ascontiguousarray(A[:, EOS_T : EOS_T + 1])
    TBT = np.ascontiguousarray(np.array(ThetaB, np.float32).T).astype(
        ml_dtypes.bfloat16
    )
    E_bf = np.asarray(E, np.float32).astype(ml_dtypes.bfloat16)
    iota = np.arange(K, dtype=np.float32).reshape(K, 1)
    ebias = np.zeros((K, 1), np.float32)
    ebias[EOS_T:, 0] = -1000.0
    alpha0 = np.zeros((K, 1), np.float32)
    alpha0[BOS_T, 0] = 1.0

    words = np.asarray(words)
    tags = np.asarray(tags)

    in_maps = []
    for c in range(NCORES):
        sl = slice(c * S, (c + 1) * S)
        wc = words[sl]  # [S, T]
        tc_ = tags[sl]
        # instance order c = (t-1)*S + s, t in 1..T-2
        widx_flat = np.ascontiguousarray(wc[:, 1 : T_ - 1].T).reshape(ni)
        tags_flat = np.ascontiguousarray(tc_[:, 1 : T_ - 1].T).reshape(ni)
        # widx shipped pre-swizzled so SBUF tile [128, nchunks] loads contiguously
        widx2 = np.ascontiguousarray(widx_flat.reshape(nchunks, 128).T).astype(np.int32)
        in_maps.append(
            {
                "E": E_bf,
                "TBT": TBT,
                "Amat": A,
                "Aeos": Aeos,
                "widx": widx2,
                "tagsf": tags_flat.astype(np.float32).reshape(1, ni),
                "iota64": iota,
                "ebias": ebias,
                "alpha0": alpha0,
            }
        )
    return in_maps


_CACHED_NC = None


def kernel(WA, ThetaB, E, words, tags):
    global _CACHED_NC
    from concourse.bass_utils import run_bass_kernel_spmd

    if _CACHED_NC is None:
        _CACHED_NC = build_bass()
    nc = _CACHED_NC
    in_maps = make_in_maps(WA, ThetaB, E, words, tags)
    res = run_bass_kernel_spmd(nc, in_maps, list(range(NCORES)))
    out = np.concatenate(
        [np.asarray(res.results[i]["out"]).reshape(S) for i in range(NCORES)]
    )
    return out.astype(np.float32)


if __name__ == "__main__":
    import reference

    inputs = {k: np.asarray(v) for k, v in reference.setup_inputs().items()}
    got = kernel(**inputs)
    print(got[:8])

